# revision 11
# baseline (speedup 1.0000x reference)
"""Trainium2 Bass kernel for Jacobi-KAN layer.

y[b,o] = sum_{i,d} P_d(tanh(x[b,i])) * C[i,o,d],  B=262144, I=O=128, D+1=9,
Jacobi polynomials with a=b=1.

Strategy (pure data parallel over batch, 8 cores):
 - Host re-expresses the degree-8 Jacobi basis in the well-conditioned
   product basis [1, t, q=t^2, t3, r=w^2, t*r, q*r, t3*r, (2r-1)^2]
   (w = 2t^2-1); folds the 9x9 change of basis into the coefficient
   tensor in float64.  The constant plane becomes a host-side bias, so
   the device does 8 accumulating matmuls per output tile instead of 9.
 - Host pre-transposes each x shard to (128, 32768) fp16.
 - Device per 2048-column chunk: ACT does tanh + 2 fused-affine squares,
   DVE does 5 fp16 multiplies (2x mode) to build the 8 basis planes;
   PE runs a d-outer/group-inner loop (weights stay loaded across the 4
   column groups) accumulating y^T into one 4-bank PSUM tile; a single
   fp32->fp16 evacuation copy per chunk alternates between ACT and DVE
   to balance engine load, then DMA to DRAM.
 - Host transposes y^T back, adds the bias vector.
"""

import sys

for _p in ("/opt/trn_rl_repo", "/opt/trn_rl_repo/concourse"):
    if _p not in sys.path:
        sys.path.insert(0, _p)

import numpy as np

import concourse.bacc as bacc
import concourse.bass as bass
import concourse.mybir as mybir
from concourse.bass_utils import run_bass_kernel_spmd
from concourse.tile import TileContext

P = 128
N_CORES = 8
B_TOTAL = 262144
B_CORE = B_TOTAL // N_CORES        # 32768
ND = 9                             # basis size (incl. constant)
NMM = 8                            # matmul planes (constant folded to bias)
CHUNK = 4096                       # elementwise chunk (free dim)
NCHUNKS = B_CORE // CHUNK          # 8
GROUP = 512                        # matmul moving free dim / PSUM bank
HALF = CHUNK // 2                  # 2048: one 4-bank PSUM tile
GROUPS_PER_HALF = HALF // GROUP    # 4

F16 = mybir.dt.float16
F32 = mybir.dt.float32
AF = mybir.ActivationFunctionType
OP = mybir.AluOpType


def _basis_transform():
    """9x9 float64 matrix T with C'[i,o,j] = sum_d C[i,o,d] * T[d,j] such
    that sum_j C'_j * plane_j(t) == sum_d C_d * JacobiP_d(t) for planes
    [1, t, q, t*q, r, t*r, q*r, t*q*r, (2r-1)^2], q=t^2, r=(2q-1)^2."""
    import numpy.polynomial.polynomial as NP

    a_, b_ = 1.0, 1.0
    polys = [np.array([1.0]), np.array([0.0, 2.0])]
    for i in range(2, ND):
        Ai = (2 * i + a_ + b_ - 1) * (2 * i + a_ + b_) / (2 * i * (i + a_ + b_))
        Bi = (2 * i + a_ + b_ - 1) * (a_ ** 2 - b_ ** 2) / (
            2 * i * (i + a_ + b_) * (2 * i + a_ + b_ - 2))
        Ci = -2 * (i + a_ - 1) * (i + b_ - 1) * (2 * i + a_ + b_) / (
            2 * i * (i + a_ + b_) * (2 * i + a_ + b_ - 2))
        p = NP.polyadd(NP.polymul([Bi, Ai], polys[i - 1]),
                       NP.polymul([Ci], polys[i - 2]))
        polys.append(p)
    Jm = np.zeros((ND, ND))
    for d, p in enumerate(polys):
        Jm[d, :len(p)] = p

    t = np.array([0.0, 1.0])
    q = NP.polymul(t, t)
    w = NP.polyadd(NP.polymul([2.0], q), [-1.0])
    r = NP.polymul(w, w)
    t2r1 = NP.polyadd(NP.polymul([2.0], r), [-1.0])
    basis = [np.array([1.0]), t, q, NP.polymul(t, q), r, NP.polymul(t, r),
             NP.polymul(q, r), NP.polymul(NP.polymul(t, q), r),
             NP.polymul(t2r1, t2r1)]
    Bm = np.zeros((ND, ND))
    for j, p in enumerate(basis):
        Bm[j, :len(p)] = p
    return Jm @ np.linalg.inv(Bm)


def _host_prep(jacobi_coeffs):
    """Returns (cw fp16 [P, 8*P] d-major, bias float64 [P])."""
    T = _basis_transform()
    Cp = np.einsum("iod,dj->ioj", jacobi_coeffs.astype(np.float64), T)
    bias = Cp[:, :, 0].sum(axis=0)                       # (128,)
    cw = np.ascontiguousarray(
        Cp[:, :, 1:].transpose(0, 2, 1).reshape(P, NMM * P)).astype(np.float16)
    return cw, bias


def prepare_in_maps(x):
    """Shard + transpose x for the 8 cores."""
    in_maps = []
    for k in range(N_CORES):
        shard = x[k * B_CORE:(k + 1) * B_CORE].astype(np.float16)
        in_maps.append({"xt": np.ascontiguousarray(shard.T)})
    return in_maps


def _build_module():
    nc = bacc.Bacc(trn_type="TRN2")
    xt = nc.dram_tensor("xt", [P, B_CORE], F16, kind="ExternalInput")
    cw = nc.dram_tensor("cw", [P, NMM * P], F16, kind="ExternalInput")
    yt = nc.dram_tensor("yt", [P, B_CORE], F16, kind="ExternalOutput")

    with TileContext(nc) as tc:
        with (
            tc.tile_pool(name="const", bufs=1) as const_pool,
            tc.tile_pool(name="xin", bufs=3) as xin_pool,
            tc.tile_pool(name="bas", bufs=2) as bas_pool,
            tc.tile_pool(name="yout", bufs=4) as yout_pool,
            tc.tile_pool(name="psum", bufs=1, space="PSUM") as psum_pool,
        ):
            cw_sb = const_pool.tile([P, NMM * P], F16)
            nc.sync.dma_start(cw_sb[:], cw[:, :])
            neg1 = const_pool.tile([P, 1], F32)
            nc.vector.memset(neg1[:], -1.0)

            for c in range(NCHUNKS):
                xin = xin_pool.tile([P, CHUNK], F16)
                t = bas_pool.tile([P, CHUNK], F16)
                q = bas_pool.tile([P, CHUNK], F16)
                t3 = bas_pool.tile([P, CHUNK], F16)
                r = bas_pool.tile([P, CHUNK], F16)
                t5 = bas_pool.tile([P, CHUNK], F16)
                qr = bas_pool.tile([P, CHUNK], F16)
                t7 = bas_pool.tile([P, CHUNK], F16)
                s8 = bas_pool.tile([P, CHUNK], F16)

                # For the first chunk, emit the producer chain split at
                # column 512 so the first matmul group's operands are ready
                # ~5us earlier (subtile deps let MMs start on the first 512
                # columns while the rest is still being computed).
                spans = [(0, 512), (512, CHUNK)] if c == 0 else [(0, CHUNK)]
                for lo, hi in spans:
                    sl = slice(lo, hi)
                    nc.sync.dma_start(xin[:, sl],
                                      xt[:, c * CHUNK + lo:c * CHUNK + hi])
                    nc.scalar.activation(t[:, sl], xin[:, sl], AF.Tanh)
                    nc.vector.tensor_tensor(q[:, sl], t[:, sl], t[:, sl],
                                            OP.mult)
                    # r = (2q-1)^2, s8 = (2r-1)^2: affine folded into ACT
                    nc.scalar.activation(r[:, sl], q[:, sl], AF.Square,
                                         bias=neg1[:], scale=2.0)
                    nc.vector.tensor_tensor(t3[:, sl], t[:, sl], q[:, sl],
                                            OP.mult)
                    nc.vector.tensor_tensor(t5[:, sl], t[:, sl], r[:, sl],
                                            OP.mult)
                    nc.vector.tensor_tensor(qr[:, sl], q[:, sl], r[:, sl],
                                            OP.mult)
                    nc.vector.tensor_tensor(t7[:, sl], t3[:, sl], r[:, sl],
                                            OP.mult)
                    nc.scalar.activation(s8[:, sl], r[:, sl], AF.Square,
                                         bias=neg1[:], scale=2.0)

                planes = [t, q, t3, r, t5, qr, t7, s8]
                # Two 4-bank PSUM tiles per chunk; evacuation of half A
                # overlaps half B's matmuls (and vice versa across chunks).
                for h, (engine, accname) in enumerate(
                        [("act", "accA"), ("dve", "accB")]):
                    base = h * HALF
                    acc = psum_pool.tile([P, HALF], F32, name=accname)
                    # d-outer / group-inner: weights reused across groups
                    for d in range(NMM):
                        lhsT = cw_sb[:, d * P:(d + 1) * P]
                        for g in range(GROUPS_PER_HALF):
                            col = base + g * GROUP
                            nc.tensor.matmul(
                                acc[:, g * GROUP:(g + 1) * GROUP], lhsT,
                                planes[d][:, col:col + GROUP],
                                start=(d == 0), stop=(d == NMM - 1))
                    yo = yout_pool.tile([P, HALF], F16)
                    last = (c == NCHUNKS - 1 and h == 1)
                    if last:
                        # split the final evacuation so the tail after the
                        # last matmul is one 1024-col copy, not 2048
                        nc.vector.tensor_copy(yo[:, :HALF // 2],
                                              acc[:, :HALF // 2])
                        nc.vector.tensor_copy(yo[:, HALF // 2:],
                                              acc[:, HALF // 2:])
                        col = c * CHUNK + base
                        nc.sync.dma_start(
                            yt[:, col:col + HALF // 2], yo[:, :HALF // 2])
                        nc.sync.dma_start(
                            yt[:, col + HALF // 2:col + HALF],
                            yo[:, HALF // 2:])
                    else:
                        if engine == "act":
                            nc.scalar.activation(yo[:], acc[:], AF.Copy)
                        else:
                            nc.vector.tensor_copy(yo[:], acc[:])
                        col = c * CHUNK + base
                        nc.sync.dma_start(yt[:, col:col + HALF], yo[:])

    # TRN2 allows at most one sync wait per instruction; split multi-wait
    # instructions into event-semaphore chains (normally done in
    # Bacc.compile(), which the bass2jax serialization path does not run).
    from concourse import inst_simplify

    nc.insert_bir_kernel_barrier_sem_inc()
    nc.move_matmul_waits_to_ldweights()
    nc.generate_event_semaphores()
    nc.remove_dead_instructions_after_branch()
    nc.validate_blocks()
    nc.dce_regs()
    nc.thread_jumps()
    nc.remove_dead_blocks()
    nc.remove_dead_allocations()
    nc.verify_switch_hints()
    nc.alloc_regs()
    inst_simplify.simplify(nc)
    nc.fuse_regops()
    nc.fuse_blocks()
    nc.replace_nops_with_events()
    for engine in nc.engines:
        nc.fuse_nops(engine)
    nc.remove_dead_nops()
    nc.remove_dangling_data()
    nc.generate_event_semaphores()
    return nc


_NC_CACHE = None


def get_module():
    global _NC_CACHE
    if _NC_CACHE is None:
        _NC_CACHE = _build_module()
    return _NC_CACHE


def postprocess(results, bias):
    out = np.concatenate(
        [np.asarray(r["yt"]).astype(np.float32).T for r in results], axis=0)
    out = out + bias[None, :].astype(np.float32)
    return np.ascontiguousarray(out.astype(np.float32))


def kernel(x: np.ndarray, jacobi_coeffs: np.ndarray) -> np.ndarray:
    x = np.asarray(x)
    C = np.asarray(jacobi_coeffs)

    cw, bias = _host_prep(C)
    in_maps = prepare_in_maps(x)
    for m in in_maps:
        m["cw"] = cw

    res = run_bass_kernel_spmd(get_module(), in_maps,
                               core_ids=list(range(N_CORES)))
    return postprocess(res.results, bias)


# revision 13
# speedup vs baseline: 1.0455x; 1.0455x over previous
"""Trainium2 Bass kernel for Jacobi-KAN layer.

y[b,o] = sum_{i,d} P_d(tanh(x[b,i])) * C[i,o,d],  B=262144, I=O=128, D+1=9,
Jacobi polynomials with a=b=1.

Strategy (pure data parallel over batch, 8 cores):
 - Host re-expresses the degree-8 Jacobi basis in the well-conditioned
   product basis [1, t, q=t^2, t3, r=w^2, t*r, q*r, t3*r, (2r-1)^2]
   (w = 2t^2-1); folds the 9x9 change of basis into the coefficient
   tensor in float64.  The constant plane becomes a host-side bias, so
   the device does 8 accumulating matmuls per output tile instead of 9.
 - Host pre-transposes each x shard to (128, 32768) fp16.
 - Device per 2048-column chunk: ACT does tanh + 2 fused-affine squares,
   DVE does 5 fp16 multiplies (2x mode) to build the 8 basis planes;
   PE runs a d-outer/group-inner loop (weights stay loaded across the 4
   column groups) accumulating y^T into one 4-bank PSUM tile; a single
   fp32->fp16 evacuation copy per chunk alternates between ACT and DVE
   to balance engine load, then DMA to DRAM.
 - Host transposes y^T back, adds the bias vector.
"""

import sys

for _p in ("/opt/trn_rl_repo", "/opt/trn_rl_repo/concourse"):
    if _p not in sys.path:
        sys.path.insert(0, _p)

import numpy as np

import concourse.bacc as bacc
import concourse.bass as bass
import concourse.mybir as mybir
from concourse.bass_utils import run_bass_kernel_spmd
from concourse.tile import TileContext

P = 128
N_CORES = 8
B_TOTAL = 262144
B_CORE = B_TOTAL // N_CORES        # 32768
ND = 9                             # basis size (incl. constant)
NMM = 8                            # matmul planes (constant folded to bias)
CHUNK = 2048                       # elementwise chunk (free dim)
NCHUNKS = B_CORE // CHUNK          # 16
GROUP = 512                        # matmul moving free dim / PSUM bank
GROUPS_PER_CHUNK = CHUNK // GROUP  # 4

F16 = mybir.dt.float16
F32 = mybir.dt.float32
AF = mybir.ActivationFunctionType
OP = mybir.AluOpType


def _basis_transform():
    """9x9 float64 matrix T with C'[i,o,j] = sum_d C[i,o,d] * T[d,j] such
    that sum_j C'_j * plane_j(t) == sum_d C_d * JacobiP_d(t) for planes
    [1, t, q, t*q, r, t*r, q*r, t*q*r, (2r-1)^2], q=t^2, r=(2q-1)^2."""
    import numpy.polynomial.polynomial as NP

    a_, b_ = 1.0, 1.0
    polys = [np.array([1.0]), np.array([0.0, 2.0])]
    for i in range(2, ND):
        Ai = (2 * i + a_ + b_ - 1) * (2 * i + a_ + b_) / (2 * i * (i + a_ + b_))
        Bi = (2 * i + a_ + b_ - 1) * (a_ ** 2 - b_ ** 2) / (
            2 * i * (i + a_ + b_) * (2 * i + a_ + b_ - 2))
        Ci = -2 * (i + a_ - 1) * (i + b_ - 1) * (2 * i + a_ + b_) / (
            2 * i * (i + a_ + b_) * (2 * i + a_ + b_ - 2))
        p = NP.polyadd(NP.polymul([Bi, Ai], polys[i - 1]),
                       NP.polymul([Ci], polys[i - 2]))
        polys.append(p)
    Jm = np.zeros((ND, ND))
    for d, p in enumerate(polys):
        Jm[d, :len(p)] = p

    t = np.array([0.0, 1.0])
    q = NP.polymul(t, t)
    w = NP.polyadd(NP.polymul([2.0], q), [-1.0])
    r = NP.polymul(w, w)
    t2r1 = NP.polyadd(NP.polymul([2.0], r), [-1.0])
    basis = [np.array([1.0]), t, q, NP.polymul(t, q), r, NP.polymul(t, r),
             NP.polymul(q, r), NP.polymul(NP.polymul(t, q), r),
             NP.polymul(t2r1, t2r1)]
    Bm = np.zeros((ND, ND))
    for j, p in enumerate(basis):
        Bm[j, :len(p)] = p
    return Jm @ np.linalg.inv(Bm)


def _host_prep(jacobi_coeffs):
    """Returns (cw fp16 [P, 8*P] d-major, bias float64 [P])."""
    T = _basis_transform()
    Cp = np.einsum("iod,dj->ioj", jacobi_coeffs.astype(np.float64), T)
    bias = Cp[:, :, 0].sum(axis=0)                       # (128,)
    cw = np.ascontiguousarray(
        Cp[:, :, 1:].transpose(0, 2, 1).reshape(P, NMM * P)).astype(np.float16)
    return cw, bias


def prepare_in_maps(x):
    """Shard + transpose x for the 8 cores."""
    in_maps = []
    for k in range(N_CORES):
        shard = x[k * B_CORE:(k + 1) * B_CORE].astype(np.float16)
        in_maps.append({"xt": np.ascontiguousarray(shard.T)})
    return in_maps


def _build_module():
    nc = bacc.Bacc(trn_type="TRN2")
    xt = nc.dram_tensor("xt", [P, B_CORE], F16, kind="ExternalInput")
    cw = nc.dram_tensor("cw", [P, NMM * P], F16, kind="ExternalInput")
    yt = nc.dram_tensor("yt", [P, B_CORE], F16, kind="ExternalOutput")

    with TileContext(nc) as tc:
        with (
            tc.tile_pool(name="const", bufs=1) as const_pool,
            tc.tile_pool(name="xin", bufs=3) as xin_pool,
            tc.tile_pool(name="bas", bufs=3) as bas_pool,
            tc.tile_pool(name="yout", bufs=3) as yout_pool,
            tc.tile_pool(name="psum", bufs=2, space="PSUM") as psum_pool,
        ):
            cw_sb = const_pool.tile([P, NMM * P], F16)
            nc.sync.dma_start(cw_sb[:], cw[:, :])
            neg1 = const_pool.tile([P, 1], F32)
            nc.vector.memset(neg1[:], -1.0)

            for c in range(NCHUNKS):
                xin = xin_pool.tile([P, CHUNK], F16)
                t = bas_pool.tile([P, CHUNK], F16)
                q = bas_pool.tile([P, CHUNK], F16)
                t3 = bas_pool.tile([P, CHUNK], F16)
                r = bas_pool.tile([P, CHUNK], F16)
                t5 = bas_pool.tile([P, CHUNK], F16)
                qr = bas_pool.tile([P, CHUNK], F16)
                t7 = bas_pool.tile([P, CHUNK], F16)
                s8 = bas_pool.tile([P, CHUNK], F16)

                # First chunk: emit the producer chain split at column 512
                # so the first matmul group's operands are ready earlier
                # (subtile deps let the PE start while the rest computes).
                spans = [(0, GROUP), (GROUP, CHUNK)] if c == 0 \
                    else [(0, CHUNK)]
                for lo, hi in spans:
                    sl = slice(lo, hi)
                    nc.sync.dma_start(xin[:, sl],
                                      xt[:, c * CHUNK + lo:c * CHUNK + hi])
                    nc.scalar.activation(t[:, sl], xin[:, sl], AF.Tanh)
                    nc.vector.tensor_tensor(q[:, sl], t[:, sl], t[:, sl],
                                            OP.mult)
                    # r = (2q-1)^2, s8 = (2r-1)^2: affine folded into ACT
                    nc.scalar.activation(r[:, sl], q[:, sl], AF.Square,
                                         bias=neg1[:], scale=2.0)
                    nc.vector.tensor_tensor(t3[:, sl], t[:, sl], q[:, sl],
                                            OP.mult)
                    nc.vector.tensor_tensor(t5[:, sl], t[:, sl], r[:, sl],
                                            OP.mult)
                    nc.vector.tensor_tensor(qr[:, sl], q[:, sl], r[:, sl],
                                            OP.mult)
                    nc.vector.tensor_tensor(t7[:, sl], t3[:, sl], r[:, sl],
                                            OP.mult)
                    nc.scalar.activation(s8[:, sl], r[:, sl], AF.Square,
                                         bias=neg1[:], scale=2.0)

                planes = [t, q, t3, r, t5, qr, t7, s8]
                acc = psum_pool.tile([P, CHUNK], F32)  # 4 PSUM banks
                # d-outer / group-inner: weights stay loaded across groups
                for d in range(NMM):
                    lhsT = cw_sb[:, d * P:(d + 1) * P]
                    for g in range(GROUPS_PER_CHUNK):
                        nc.tensor.matmul(
                            acc[:, g * GROUP:(g + 1) * GROUP], lhsT,
                            planes[d][:, g * GROUP:(g + 1) * GROUP],
                            start=(d == 0), stop=(d == NMM - 1))
                # evacuate fp32 PSUM -> fp16 SBUF, alternating engines to
                # balance ACT/DVE load, then DMA to DRAM
                yo = yout_pool.tile([P, CHUNK], F16)
                if c == NCHUNKS - 1:
                    # split the final evacuation so the tail after the last
                    # matmul is one 1024-col copy + small DMA, not 2048
                    h = CHUNK // 2
                    nc.vector.tensor_copy(yo[:, :h], acc[:, :h])
                    nc.vector.tensor_copy(yo[:, h:], acc[:, h:])
                    col = c * CHUNK
                    nc.sync.dma_start(yt[:, col:col + h], yo[:, :h])
                    nc.sync.dma_start(yt[:, col + h:col + CHUNK], yo[:, h:])
                elif c % 2 == 0:
                    nc.scalar.activation(yo[:], acc[:], AF.Copy)
                    nc.sync.dma_start(yt[:, c * CHUNK:(c + 1) * CHUNK], yo[:])
                else:
                    nc.vector.tensor_copy(yo[:], acc[:])
                    nc.sync.dma_start(yt[:, c * CHUNK:(c + 1) * CHUNK], yo[:])

    # TRN2 allows at most one sync wait per instruction; split multi-wait
    # instructions into event-semaphore chains (normally done in
    # Bacc.compile(), which the bass2jax serialization path does not run).
    from concourse import inst_simplify

    nc.insert_bir_kernel_barrier_sem_inc()
    nc.move_matmul_waits_to_ldweights()
    nc.generate_event_semaphores()
    nc.remove_dead_instructions_after_branch()
    nc.validate_blocks()
    nc.dce_regs()
    nc.thread_jumps()
    nc.remove_dead_blocks()
    nc.remove_dead_allocations()
    nc.verify_switch_hints()
    nc.alloc_regs()
    inst_simplify.simplify(nc)
    nc.fuse_regops()
    nc.fuse_blocks()
    nc.replace_nops_with_events()
    for engine in nc.engines:
        nc.fuse_nops(engine)
    nc.remove_dead_nops()
    nc.remove_dangling_data()
    nc.generate_event_semaphores()
    return nc


_NC_CACHE = None


def get_module():
    global _NC_CACHE
    if _NC_CACHE is None:
        _NC_CACHE = _build_module()
    return _NC_CACHE


def postprocess(results, bias):
    out = np.concatenate(
        [np.asarray(r["yt"]).astype(np.float32).T for r in results], axis=0)
    out = out + bias[None, :].astype(np.float32)
    return np.ascontiguousarray(out.astype(np.float32))


def kernel(x: np.ndarray, jacobi_coeffs: np.ndarray) -> np.ndarray:
    x = np.asarray(x)
    C = np.asarray(jacobi_coeffs)

    cw, bias = _host_prep(C)
    in_maps = prepare_in_maps(x)
    for m in in_maps:
        m["cw"] = cw

    res = run_bass_kernel_spmd(get_module(), in_maps,
                               core_ids=list(range(N_CORES)))
    return postprocess(res.results, bias)
